# revision 6
# baseline (speedup 1.0000x reference)
"""Center-loss kernel for Trainium2 (8 NeuronCores, Bass/Tile).

Math reduction: with n_c = per-class count and S_c = segment-sum of feature
rows by label,
    loss        = 0.5*sum|f|^2 - sum_c <S_c, centers_c> + 0.5*sum_c n_c*|centers_c|^2
    new_centers = centers + ALPHA/(1+n) * (S - n*centers)
so the device only needs S (segment sums) and sum|f|^2 (via a Gram-matrix
accumulation); counts come from labels on the host.

Device algorithm (per core, data-parallel over rows):
  rows are gathered in label-sorted order (host-computed permutation, int16
  indices over 32768-row sub-shards) with `dma_gather`; each 512-row tile's
  labels then fall in a narrow class window [base, base+128), so the
  segment-sum of the tile is a 128x128 one-hot matmul accumulated in PSUM.
  Window results (S^T slices) and the Gram matrix stream back to DRAM; the
  host adds windows into S at their bases, combines the 8 cores, and applies
  the closed-form update above.
"""
import numpy as np
import ml_dtypes

import concourse.bacc as bacc
import concourse.bass as bass
import concourse.mybir as mybir
import concourse.tile as tile
from concourse.bass_utils import run_bass_kernel_spmd

P = 128            # partitions / feature dim
ALPHA = 0.5
NUM_CLASSES = 7000
FEAT_DIM = 128
N_ROWS = 524288
N_CORES = 8
R = 512            # rows per tile
G = R // P         # matmul sub-tiles per tile
W = 128            # class window width per tile
SUB = 32768        # rows per sub-shard (int16 index range)
K_CALL = 4         # tiles per dma_gather call

_BF16 = ml_dtypes.bfloat16


# ----------------------------------------------------------------- host plan

def _plan_subshard(labels_sub: np.ndarray):
    """Greedy window tiling of one sub-shard's sorted rows.

    Returns (idx [T,R] int32 row ids (pads->0), jcol [T,R] f32 (pads->-1),
    bases [T] int64, pad_count int).
    """
    n = labels_sub.shape[0]
    order = np.argsort(labels_sub, kind="stable")
    slab = labels_sub[order]
    idx_rows, jcol_rows, bases = [], [], []
    pad_count = 0
    pos = 0
    while pos < n:
        base = int(slab[pos])
        hi = int(np.searchsorted(slab, base + W, side="left"))
        end = min(pos + R, hi)
        rows = order[pos:end].astype(np.int32)
        j = (slab[pos:end] - base).astype(np.float32)
        pad = R - (end - pos)
        if pad:
            pad_count += pad
            rows = np.concatenate([rows, np.zeros(pad, np.int32)])
            j = np.concatenate([j, np.full(pad, -1.0, np.float32)])
        idx_rows.append(rows)
        jcol_rows.append(j)
        bases.append(base)
        pos = end
    return (np.stack(idx_rows), np.stack(jcol_rows),
            np.array(bases, dtype=np.int64), pad_count)


def _pad_tiles(idx_t, jcol_t, bases, T_pad):
    """Extend to T_pad tiles with full pads (row 0, jcol=-1, base=0)."""
    T = idx_t.shape[0]
    pad_rows = 0
    if T < T_pad:
        pad_rows = (T_pad - T) * R
        idx_t = np.concatenate([idx_t, np.zeros((T_pad - T, R), np.int32)])
        jcol_t = np.concatenate([jcol_t, np.full((T_pad - T, R), -1.0, np.float32)])
        bases = np.concatenate([bases, np.zeros(T_pad - T, np.int64)])
    return idx_t, jcol_t, bases, pad_rows


def _pack_core(idx_subs, jcol_subs, T_sub, sub=SUB):
    """Pack per-sub-shard [T_sub, R] plans into device arrays.

    idx16  [P, n_sub*T_sub*R/16] int16 wrapped in 16 partitions and
           replicated to 128; logical gather order i = sorted position.
    consts [P, n_sub*T_sub*G + P] bf16: per-tile jcol columns then iota.
    """
    n_sub = len(idx_subs)
    # int16 gather indices in logical order, per sub-shard
    idx_flat = np.concatenate([s.reshape(-1) for s in idx_subs])  # [n_sub*T_sub*R]
    assert idx_flat.max() < min(sub, 32768) and idx_flat.min() >= 0
    idx16 = idx_flat.astype(np.int16)
    free = idx16.shape[0] // 16
    wrapped = idx16.reshape(free, 16).T                    # [16, free]
    idx_dev = np.ascontiguousarray(np.tile(wrapped, (8, 1)))  # [128, free]

    jcol_flat = np.concatenate([s for s in jcol_subs])      # [n_sub*T_sub, R]
    TT = jcol_flat.shape[0]
    jcol_p = jcol_flat.reshape(TT, G, P).transpose(2, 0, 1).reshape(P, TT * G)
    iota = np.broadcast_to(np.arange(P, dtype=np.float32), (P, P))
    consts = np.concatenate([jcol_p, iota], axis=1).astype(_BF16)
    return idx_dev, np.ascontiguousarray(consts)


# ------------------------------------------------------------- device build

def _build_nc(n_shard, n_sub_rows, n_sub, T_sub, k_call):
    """Build the per-core Bass program (SPMD: same program, per-core data)."""
    assert T_sub % k_call == 0
    TT = n_sub * T_sub
    calls_per_sub = T_sub // k_call
    rows_call = k_call * R
    idx_free_per_call = rows_call // 16

    nc = bacc.Bacc("TRN2")
    feat = nc.dram_tensor("feat", [n_shard, P], mybir.dt.float32,
                          kind="ExternalInput").ap()
    idxd = nc.dram_tensor("idx16", [P, TT * R // 16], mybir.dt.int16,
                          kind="ExternalInput").ap()
    constsd = nc.dram_tensor("consts", [P, TT * G + P], mybir.dt.bfloat16,
                             kind="ExternalInput").ap()
    winsT = nc.dram_tensor("winsT", [P, (TT + 1) * P], mybir.dt.float32,
                           kind="ExternalOutput").ap()

    with tile.TileContext(nc) as tc:
        with (
            tc.tile_pool(name="const", bufs=1) as constp,
            tc.tile_pool(name="fw", bufs=3) as fwp,
            tc.tile_pool(name="fb", bufs=3) as fbp,
            tc.tile_pool(name="oh", bufs=4) as ohp,
            tc.tile_pool(name="ps", bufs=4, space="PSUM") as psp,
            tc.tile_pool(name="psg", bufs=1, space="PSUM") as psgp,
        ):
            idx_sb = constp.tile([P, TT * R // 16], mybir.dt.int16)
            consts_sb = constp.tile([P, TT * G + P], mybir.dt.bfloat16)
            so_buf = constp.tile([P, (TT + 1) * P], mybir.dt.float32)
            nc.sync.dma_start(out=idx_sb[:], in_=idxd[:])
            nc.sync.dma_start(out=consts_sb[:], in_=constsd[:])
            jcol_sb = consts_sb[:, 0:TT * G]
            iota_sb = consts_sb[:, TT * G:]

            psG = psgp.tile([P, P], mybir.dt.float32)

            for h in range(n_sub):
                src = feat[h * n_sub_rows:(h + 1) * n_sub_rows, :]
                for c in range(calls_per_sub):
                    call_i = h * calls_per_sub + c
                    ftw = fwp.tile([P, k_call * G, P], mybir.dt.float32, tag="ftw")
                    nc.gpsimd.dma_gather(
                        out_ap=ftw[:, :, :],
                        in_ap=src,
                        idxs_ap=idx_sb[:, call_i * idx_free_per_call:
                                       (call_i + 1) * idx_free_per_call],
                        num_idxs=rows_call,
                        num_idxs_reg=rows_call,
                        elem_size=P,
                        single_packet=(rows_call <= 1024),
                    )
                    ftb = fbp.tile([P, k_call * G, P], mybir.dt.bfloat16, tag="ftb")
                    nc.scalar.copy(out=ftb[:, :, :], in_=ftw[:, :, :])

                    for k in range(k_call):
                        t = call_i * k_call + k
                        oh = ohp.tile([P, G, P], mybir.dt.bfloat16, tag="oh")
                        io3 = bass.AP(iota_sb.tensor, iota_sb.offset,
                                      [iota_sb.ap[0], [0, G], iota_sb.ap[1]])
                        jc = jcol_sb[:, t * G:(t + 1) * G]
                        j3 = bass.AP(jc.tensor, jc.offset,
                                     [jc.ap[0], jc.ap[1], [0, P]])
                        nc.vector.tensor_tensor(
                            out=oh[:, :, :], in0=io3, in1=j3,
                            op=mybir.AluOpType.is_equal,
                        )

                        psS = psp.tile([P, P], mybir.dt.float32, tag="psS")
                        for g in range(G):
                            nc.tensor.matmul(
                                out=psS[:],
                                lhsT=ftb[:, k * G + g, :],
                                rhs=oh[:, g, :],
                                start=(g == 0),
                                stop=(g == G - 1),
                            )
                        for g in range(G):
                            nc.tensor.matmul(
                                out=psG[:],
                                lhsT=ftb[:, k * G + g, :],
                                rhs=ftb[:, k * G + g, :],
                                start=(t == 0 and g == 0),
                                stop=(t == TT - 1 and g == G - 1),
                                skip_group_check=True,
                            )

                        nc.vector.tensor_copy(out=so_buf[:, t * P:(t + 1) * P],
                                              in_=psS[:])

            nc.vector.tensor_copy(out=so_buf[:, TT * P:], in_=psG[:])
            nc.sync.dma_start(out=winsT[:], in_=so_buf[:])
    nc.compile()
    return nc


_NC_CACHE = {}


def _get_nc(n_shard, n_sub_rows, n_sub, T_sub, k_call):
    key = (n_shard, n_sub_rows, n_sub, T_sub, k_call)
    if key not in _NC_CACHE:
        _NC_CACHE[key] = _build_nc(*key)
    return _NC_CACHE[key]


# ------------------------------------------------------------------ wrapper

def _prepare(features, labels, n_cores=N_CORES, sub=SUB, k_call=K_CALL):
    """Host planning for all cores. Returns in_maps, per-core metadata, T_sub."""
    N = features.shape[0]
    n_shard = N // n_cores
    n_sub = n_shard // sub
    plans = []  # per core: list of (idx_t, jcol_t, bases, pad_count)
    for c in range(n_cores):
        lab = labels[c * n_shard:(c + 1) * n_shard]
        plans.append([_plan_subshard(lab[h * sub:(h + 1) * sub])
                      for h in range(n_sub)])
    T_max = max(p[0].shape[0] for core in plans for p in core)
    T_sub = -(-T_max // k_call) * k_call  # round up to k_call multiple

    in_maps, metas = [], []
    for c in range(n_cores):
        idx_subs, jcol_subs, bases_subs, pads_subs = [], [], [], []
        for h in range(n_sub):
            idx_t, jcol_t, bases, pad_c = plans[c][h]
            idx_t, jcol_t, bases, pad_rows = _pad_tiles(idx_t, jcol_t, bases, T_sub)
            idx_subs.append(idx_t)
            jcol_subs.append(jcol_t)
            bases_subs.append(bases)
            pads_subs.append(pad_c + pad_rows)
        idx_dev, consts = _pack_core(idx_subs, jcol_subs, T_sub, sub=sub)
        shard = np.ascontiguousarray(features[c * n_shard:(c + 1) * n_shard])
        in_maps.append({"feat": shard, "idx16": idx_dev, "consts": consts})
        metas.append({"bases": bases_subs, "pads": pads_subs})
    return in_maps, metas, T_sub, n_shard, n_sub


def kernel(features, labels, centers, *, _n_cores=N_CORES, _sub=SUB,
           _k_call=K_CALL, _runner=None):
    features = np.asarray(features, dtype=np.float32)
    labels = np.asarray(labels, dtype=np.int32)
    centers = np.asarray(centers, dtype=np.float32)
    N, D = features.shape
    C = centers.shape[0]

    in_maps, metas, T_sub, n_shard, n_sub = _prepare(
        features, labels, n_cores=_n_cores, sub=_sub, k_call=_k_call)
    TT = n_sub * T_sub
    nc = _get_nc(n_shard, _sub, n_sub, T_sub, _k_call)

    if _runner is None:
        res = run_bass_kernel_spmd(nc, in_maps, list(range(_n_cores)))
    else:
        res = _runner(nc, in_maps)

    S_T = np.zeros((P, C + W), dtype=np.float64)
    sumsq = 0.0
    for c in range(_n_cores):
        w = res.results[c]["winsT"].reshape(P, TT + 1, P)
        gram = w[:, TT, :]
        sumsq += float(np.trace(gram))
        shard = in_maps[c]["feat"]
        for h in range(n_sub):
            bases = metas[c]["bases"][h]
            pads = metas[c]["pads"][h]
            if pads:
                f0 = shard[h * _sub].astype(_BF16).astype(np.float64)
                sumsq -= pads * float(np.dot(f0, f0))
            for t_local, base in enumerate(bases):
                t = h * T_sub + t_local
                S_T[:, base:base + W] += w[:, t, :]

    S = S_T.T[:C]                                    # [C, D] float64
    counts = np.bincount(labels, minlength=C).astype(np.float64)
    c64 = centers.astype(np.float64)
    loss = (0.5 * sumsq
            - float(np.sum(S * c64))
            + 0.5 * float(np.dot(counts, np.einsum("cd,cd->c", c64, c64))))
    new_centers = c64 + (ALPHA / (1.0 + counts))[:, None] * (S - counts[:, None] * c64)
    return np.float32(loss), new_centers.astype(np.float32)
